# revision 1
# baseline (speedup 1.0000x reference)
"""Multi-head attention on 8 TRN2 NeuronCores (Bass/Tile).

Sharding: core c handles batch b = c//2 and query-half h = c%2 (1024 query
tokens), all 16 heads. K/V projections are per-batch and duplicated across
the two cores sharing a batch, so no cross-core communication is needed;
the host splits inputs and concatenates outputs.

Design notes:
- Every on-chip tensor keeps tokens on the free axis and embed/head_dim on
  partitions. Q/K projections then directly produce the Q^T/K^T tiles the
  energy matmul wants, and the output projection directly produces Y^T
  (transposed back on the host).
- Keys are compacted on the host using the 0/1 key mask (masked keys dropped,
  padded to a multiple of 128). Pad positions are killed inside the softmax
  by a -1e9 per-partition bias folded into the Exp activation. This halves
  the attention and K/V projection work for a ~half-zero mask.
- Energy is computed transposed ([key, query] tiles) so the softmax
  normalizer can ride the attention@V matmul: each head's V tile carries an
  extra ones column, so the AV matmul (M=65) yields 64 output rows plus the
  softmax denominator row. Normalization is a reciprocal, a K=1 ones-matmul
  partition broadcast, and one elementwise multiply per head, deferred into
  the next head pair's pipeline so it never stalls the in-order PE queue.
- Matmul operand tensors are typed float32r (TF32-like), streaming at 1
  cycle/row vs fp32's 4, with fp32 PSUM accumulation (end-to-end relative
  error ~4e-4 vs the fp32 reference).
- Q/K projections split the contraction into two k-halves so only half the
  input chunks are SBUF-resident at a time, keeping the input DMA stream
  busy from t=0; phases C (V-proj), D (attention), and E (out-proj) share
  one PSUM pool so the Tile scheduler overlaps them by dataflow.
"""

import sys

sys.path.insert(0, "/opt/trn_rl_repo")

from contextlib import ExitStack

import numpy as np

import concourse.bass as bass  # noqa: F401  (engine types via nc)
import concourse.tile as tile
from concourse import bacc, mybir
from concourse.bass_utils import run_bass_kernel_spmd

E = 1024          # embed dim
HEADS = 16
HD = 64           # head dim
B = 4
S = 2048
NCORES = 8
Q = (B * S) // NCORES  # query tokens per core
EC = E // 128     # embed chunks of 128
F32 = mybir.dt.float32
F32R = mybir.dt.float32r


def _nchunks(total, pref=512, minsz=256):
    """Split `total` (multiple of 128, >=256) into chunks in [minsz, pref]."""
    out, rem = [], total
    while rem > 0:
        c = min(pref, rem)
        if rem - c != 0 and rem - c < minsz:
            c = rem - minsz
        out.append(c)
        rem -= c
    return out


def _starts(chunks):
    s, out = 0, []
    for c in chunks:
        out.append((s, c))
        s += c
    return out


def build_program(Kpad):
    """Build the per-core Bass program (identical on all 8 cores)."""
    KTn = Kpad // 128
    nc = bacc.Bacc("TRN2", target_bir_lowering=False, debug=False,
                   num_devices=NCORES, dynamic_dma_scratch_size=2048)

    qT = nc.dram_tensor("qT", [E, Q], F32R, kind="ExternalInput").ap()
    kT = nc.dram_tensor("kT", [E, Kpad], F32R, kind="ExternalInput").ap()
    vT = nc.dram_tensor("vT", [E, Kpad], F32R, kind="ExternalInput").ap()
    wqT = nc.dram_tensor("wqT", [E, E], F32R, kind="ExternalInput").ap()
    wkT = nc.dram_tensor("wkT", [E, E], F32R, kind="ExternalInput").ap()
    wvT = nc.dram_tensor("wvT", [E, E], F32R, kind="ExternalInput").ap()
    woT = nc.dram_tensor("woT", [E, E], F32R, kind="ExternalInput").ap()
    bq2 = nc.dram_tensor("bq2", [128, EC], F32, kind="ExternalInput").ap()
    bk2 = nc.dram_tensor("bk2", [128, EC], F32, kind="ExternalInput").ap()
    bo2 = nc.dram_tensor("bo2", [128, EC], F32, kind="ExternalInput").ap()
    bv2 = nc.dram_tensor("bv2", [1, E], F32R, kind="ExternalInput").ap()
    mb = nc.dram_tensor("mb", [128, KTn], F32, kind="ExternalInput").ap()
    onesd = nc.dram_tensor("onesd", [128, 128], F32R, kind="ExternalInput").ap()
    yT = nc.dram_tensor("yT", [E, Q], F32, kind="ExternalOutput").ap()

    q_chunks = _starts(_nchunks(Q))
    k_chunks = _starts(_nchunks(Kpad))

    with tile.TileContext(nc) as tc, ExitStack() as ctx:
        wgt = ctx.enter_context(tc.tile_pool(name="wgt", bufs=8))
        big = ctx.enter_context(tc.tile_pool(name="big", bufs=1))
        sml = ctx.enter_context(tc.tile_pool(name="sml", bufs=1))
        ptp = ctx.enter_context(tc.tile_pool(name="ptp", bufs=6))
        inp_ctx = ExitStack()
        inp = inp_ctx.enter_context(tc.tile_pool(name="inp", bufs=12))

        def load_chunks(x_dram, w_dram, ntot, xname):
            xc, wc = [], []
            for k in range(EC):
                t = inp.tile([128, ntot], F32R, tag="ic", name=f"{xname}{k}")
                nc.sync.dma_start(t[:], x_dram[k * 128:(k + 1) * 128, :])
                xc.append(t)
                w = wgt.tile([128, E], F32R, tag="wc", name=f"w{xname}{k}")
                nc.sync.dma_start(w[:], w_dram[k * 128:(k + 1) * 128, :])
                wc.append(w)
            return xc, wc

        # ---- small constants -------------------------------------------
        bq_s = sml.tile([128, EC], F32, name="bq_s")
        nc.gpsimd.dma_start(bq_s[:], bq2[:])
        bk_s = sml.tile([128, EC], F32, name="bk_s")
        nc.gpsimd.dma_start(bk_s[:], bk2[:])
        bo_s = sml.tile([128, EC], F32, name="bo_s")
        nc.gpsimd.dma_start(bo_s[:], bo2[:])
        bv_s = sml.tile([1, E], F32R, name="bv_s")
        nc.gpsimd.dma_start(bv_s[:], bv2[:])
        mb_s = sml.tile([128, KTn], F32, name="mb_s")
        nc.gpsimd.dma_start(mb_s[:], mb[:])
        ones_s = sml.tile([128, 128], F32R, name="ones_s")
        nc.gpsimd.dma_start(ones_s[:], onesd[:])

        def projh(dst_tiles, x_dram, w_dram, ntot, chunks, bias_s, xname):
            """dst[m] = (W @ X^T + b), contraction split into two k-halves
            so only 4 input chunks are resident per pass (the DMA stream
            for the next pass/phase overlaps compute). One PSUM tile per
            n-chunk (matmul out cannot cross a 512-float bank boundary);
            the second half accumulates into SBUF via a DVE add."""
            for half in (0, 1):
                xc, wc = [], []
                for kk in range(EC // 2):
                    k = half * (EC // 2) + kk
                    t = inp.tile([128, ntot], F32R, tag="ic",
                                 name=f"{xname}{k}")
                    nc.sync.dma_start(t[:], x_dram[k * 128:(k + 1) * 128, :])
                    xc.append(t)
                    w = wgt.tile([128, E], F32R, tag="wc", name=f"w{xname}{k}")
                    nc.sync.dma_start(w[:], w_dram[k * 128:(k + 1) * 128, :])
                    wc.append(w)
                for m in range(EC):
                    for i, (n0, nn) in enumerate(chunks):
                        ps = psD.tile([128, nn], F32, tag="po",
                                      name=f"ps{xname}{half}_{m}_{i}")
                        for kk in range(EC // 2):
                            nc.tensor.matmul(
                                ps[:, 0:nn],
                                wc[kk][:, m * 128:(m + 1) * 128],
                                xc[kk][:, n0:n0 + nn],
                                start=(kk == 0), stop=(kk == EC // 2 - 1))
                        if half == 0:
                            nc.vector.tensor_scalar_add(
                                dst_tiles[m][:, n0:n0 + nn],
                                ps[:, 0:nn], bias_s[:, m:m + 1])
                        else:
                            nc.vector.tensor_add(
                                dst_tiles[m][:, n0:n0 + nn],
                                dst_tiles[m][:, n0:n0 + nn], ps[:, 0:nn])

        QTs = [big.tile([128, Q], F32R, name=f"QT{m}", tag=f"QT{m}")
               for m in range(EC)]
        KTs = [big.tile([128, Kpad], F32R, name=f"KT{m}", tag=f"KT{m}")
               for m in range(EC)]
        VVs = [big.tile([128, HEADS * 65], F32R, name=f"VV{t}", tag=f"VV{t}")
               for t in range(KTn)]

        # ---- one PSUM pool for everything (pe:4 + po:4 banks) so the
        # scheduler can overlap projections, attention, and out-proj.
        psD = ctx.enter_context(tc.tile_pool(name="psD", bufs=2, space="PSUM"))

        # ---- phases A/B: Q and K projections ---------------------------
        projh(QTs, qT, wqT, Q, q_chunks, bq_s, "q")
        projh(KTs, kT, wkT, Kpad, k_chunks, bk_s, "k")

        # ---- phase C: V projection, emitted interleaved into pair 0 ----
        vc, wvc = load_chunks(vT, wvT, Kpad, "v")

        # prefetch output-projection weights during phase D
        woc = []
        for k in range(EC):
            w = wgt.tile([128, E], F32R, tag="wc", name=f"wo{k}")
            nc.sync.dma_start(w[:], woT[k * 128:(k + 1) * 128, :])
            woc.append(w)

        def emit_vtile(t):
            """V-projection tile t -> VVs[t] ([token, 16*(64+ones)])."""
            ps = psD.tile([128, E], F32, tag="pe", name=f"psv{t}")
            for n0 in (0, 512):
                for k in range(EC):
                    nc.tensor.matmul(
                        ps[:, n0:n0 + 512],
                        vc[k][:, t * 128:(t + 1) * 128],
                        wvc[k][:, n0:n0 + 512],
                        start=(k == 0), stop=False)
                # bias via K=1 ones row: V += 1 * bv
                nc.tensor.matmul(
                    ps[:, n0:n0 + 512],
                    ones_s[0:1, 0:128],
                    bv_s[0:1, n0:n0 + 512],
                    start=False, stop=True)
            vv3 = VVs[t][:].rearrange("p (h e) -> p h e", e=65)
            ps3 = ps[:].rearrange("p (h d) -> p h d", d=64)
            nc.vector.tensor_copy(vv3[:, :, 0:64], ps3[:])
            nc.vector.tensor_copy(
                vv3[:, :, 64:65],
                ones_s[:, 0:16].rearrange("p (a b) -> p a b", b=1))

        nrm = None  # opened once the input-chunk pool is released

        def make_norm(j, hh, ocp):
            """Deferred per-head normalize: out = ocp[0:64] / ocp[64]."""
            def emit():
                rc = nrm.tile([65, Q], F32R, tag="s", name=f"rc{j}_{hh}")
                with nc.allow_low_precision(
                        reason="f32r recip feeds f32r matmul broadcast"):
                    nc.vector.reciprocal(rc[64:65, :], ocp[64:65, :])
                bc_ps = psD.tile([64, Q], F32, tag="pe", name=f"bp{j}_{hh}")
                for n0 in (0, 512):
                    nc.tensor.matmul(bc_ps[0:64, n0:n0 + 512],
                                     ones_s[64:65, 0:64],
                                     rc[64:65, n0:n0 + 512],
                                     start=True, stop=True)
                bc = nrm.tile([64, Q], F32, tag="s", name=f"bc{j}_{hh}")
                nc.vector.tensor_copy(bc[:], bc_ps[:])
                if hh == 0:
                    nc.vector.tensor_mul(QTs[j][0:64, :], ocp[0:64, :],
                                         bc[0:64, :])
                else:
                    tmp = nrm.tile([64, Q], F32R, tag="s", name=f"tm{j}")
                    nc.vector.tensor_mul(tmp[:], ocp[0:64, :], bc[0:64, :])
                    # partition shift 0:64 -> 64:128 via SBUF-SBUF DMA
                    nc.sync.dma_start(QTs[j][64:128, :], tmp[:])
            return emit

        # ---- phase D: attention per head pair --------------------------
        def emit_e_exp(j, kt):
            """Energy matmuls (row-packed head pair) + masked exp."""
            pe = []
            for hh in (0, 1):
                pe.append(psD.tile([128, Q], F32, tag="pe",
                                   name=f"pe{j}_{kt}_{hh}"))
            for n0 in (0, 512):
                for hh in (0, 1):  # adjacent => row-group overlap
                    off = hh * 64
                    nc.tensor.matmul(
                        pe[hh][:, n0:n0 + 512],
                        KTs[j][off:off + 64, kt * 128:(kt + 1) * 128],
                        QTs[j][off:off + 64, n0:n0 + 512])
            pt = []
            for hh in (0, 1):
                t = ptp.tile([128, Q], F32R, tag="pt",
                             name=f"pt{j}_{kt}_{hh}")
                nc.scalar.activation(
                    t[:], pe[hh][:], mybir.ActivationFunctionType.Exp,
                    bias=mb_s[:, kt:kt + 1], scale=0.125)
                pt.append(t)
            return pt

        def emit_av(j, kt, po, pt):
            for n0 in (0, 512):
                for hh in (0, 1):
                    h = 2 * j + hh
                    nc.tensor.matmul(
                        po[hh][0:65, n0:n0 + 512],
                        VVs[kt][:, h * 65:(h + 1) * 65],
                        pt[hh][:, n0:n0 + 512],
                        start=(kt == 0), stop=(kt == KTn - 1))

        pending = []  # previous pair's deferred normalizes
        for j in range(EC):  # head pair j -> heads 2j (rows 0:64), 2j+1
            po = []
            for hh in (0, 1):
                po.append(psD.tile([65, Q], F32, tag="po",
                                   name=f"po{j}_{hh}"))
            for kt in range(KTn):
                if j == 0:
                    emit_vtile(kt)
                pt = emit_e_exp(j, kt)
                emit_av(j, kt, po, pt)
                # previous pair's normalize, deep in this pipeline
                if pending and kt >= 1:
                    pending.pop(0)()
            while pending:
                pending.pop(0)()
            if j == 0:
                # input-chunk pool is dead once V tiles are emitted
                inp_ctx.close()
                nrm = ctx.enter_context(tc.tile_pool(name="nrm", bufs=8))
            # free the po slots quickly: copy [out|denominator] to SBUF and
            # defer the actual normalize into the next pair's pipeline.
            for hh in (0, 1):
                ocp = nrm.tile([65, Q], F32, tag="s", name=f"oc{j}_{hh}")
                nc.vector.tensor_copy(ocp[:], po[hh][0:65, :])
                pending.append(make_norm(j, hh, ocp))
        while pending:
            pending.pop(0)()

        # ---- phase E: output projection Y^T = Wo @ O^T + bo ------------
        for m in range(EC):
            ps = psD.tile([128, Q], F32, tag="pe", name=f"psy{m}")
            for n0, nn in q_chunks:
                for k in range(EC):
                    nc.tensor.matmul(
                        ps[:, n0:n0 + nn],
                        woc[k][:, m * 128:(m + 1) * 128],
                        QTs[k][:, n0:n0 + nn],
                        start=(k == 0), stop=(k == EC - 1))
            yt = nrm.tile([128, Q], F32, tag="s", name=f"yt{m}")
            nc.vector.tensor_scalar_add(yt[:], ps[:], bo_s[:, m:m + 1])
            nc.sync.dma_start(yT[m * 128:(m + 1) * 128, :], yt[:])

    nc.compile()
    return nc


_ONES128 = np.ones((128, 128), np.float32)

_PROG_CACHE = {}


def _get_program(Kpad):
    key = Kpad
    if key not in _PROG_CACHE:
        _PROG_CACHE[key] = build_program(Kpad)
    return _PROG_CACHE[key]


def prepare_inputs(query, keys, values, mask, Wq, bq, Wk, bk, Wv, bv, Wo, bo):
    """Host-side sharding/layout prep. Returns (Kpad, in_maps)."""
    f32 = np.float32
    query = np.asarray(query, f32)
    keys = np.asarray(keys, f32)
    values = np.asarray(values, f32)
    mask = np.asarray(mask)

    idxs = [np.nonzero(mask[b] != 0)[0] for b in range(B)]
    nmax = max(len(i) for i in idxs)
    Kpad = max(256, ((max(nmax, 1) + 127) // 128) * 128)
    KTn = Kpad // 128

    kTb = np.zeros((B, E, Kpad), f32)
    vTb = np.zeros((B, E, Kpad), f32)
    mbb = np.full((B, Kpad), -1e9, f32)
    for b in range(B):
        n = len(idxs[b])
        kTb[b, :, :n] = keys[b][idxs[b]].T
        vTb[b, :, :n] = values[b][idxs[b]].T
        mbb[b, :n] = 0.0
    mb2 = np.ascontiguousarray(mbb.reshape(B, KTn, 128).transpose(0, 2, 1))

    WqT = np.ascontiguousarray(np.asarray(Wq, f32).T)
    WkT = np.ascontiguousarray(np.asarray(Wk, f32).T)
    WvT = np.ascontiguousarray(np.asarray(Wv, f32).T)
    WoT = np.ascontiguousarray(np.asarray(Wo, f32).T)
    bq2 = np.ascontiguousarray(np.asarray(bq, f32).reshape(EC, 128).T)
    bk2 = np.ascontiguousarray(np.asarray(bk, f32).reshape(EC, 128).T)
    bo2 = np.ascontiguousarray(np.asarray(bo, f32).reshape(EC, 128).T)
    bv2 = np.ascontiguousarray(np.asarray(bv, f32).reshape(1, E))

    in_maps = []
    for c in range(NCORES):
        b, h = c // 2, c % 2
        in_maps.append(dict(
            qT=np.ascontiguousarray(query[b, h * Q:(h + 1) * Q, :].T),
            kT=kTb[b], vT=vTb[b], mb=mb2[b],
            wqT=WqT, wkT=WkT, wvT=WvT, woT=WoT,
            bq2=bq2, bk2=bk2, bo2=bo2, bv2=bv2,
            onesd=_ONES128,
        ))
    return Kpad, in_maps


def kernel(query, keys, values, mask, Wq, bq, Wk, bk, Wv, bv, Wo, bo):
    Kpad, in_maps = prepare_inputs(query, keys, values, mask,
                                   Wq, bq, Wk, bk, Wv, bv, Wo, bo)
    nc = _get_program(Kpad)
    res = run_bass_kernel_spmd(nc, in_maps, list(range(NCORES)))
    out = np.empty((B, S, E), np.float32)
    for c in range(NCORES):
        b, h = c // 2, c % 2
        out[b, h * Q:(h + 1) * Q, :] = res.results[c]["yT"].T
    return out



# revision 6
# speedup vs baseline: 1.0304x; 1.0304x over previous
"""Multi-head attention on 8 TRN2 NeuronCores (Bass/Tile).

Sharding: core c handles batch b = c//2 and query-half h = c%2 (1024 query
tokens), all 16 heads. K/V projections are per-batch and duplicated across
the two cores sharing a batch; no cross-core communication.

Design notes (v3):
- All matmul operands are bf16 (1 PE cycle/row at any output width, vs
  fp32r's 4x penalty below 256), halving DMA traffic as well. PSUM stays
  fp32.
- Keys are compacted on the host using the 0/1 key mask (masked keys
  dropped, padded to a multiple of 128, Kpad). Pad positions are killed by
  a -1e9 per-partition bias folded into the Exp activation.
- The V-projection bias is folded into the output-projection bias on the
  host (bo' = bo + Wo @ bv), since softmax weights sum to 1.
- Energy is computed transposed ([key, query] tiles). AV is computed as
  out[q-tile, 65] accumulating over key tiles: each head's V tile carries
  an extra ones column, so column 64 of the PSUM output is the softmax
  denominator, a per-partition scalar. Normalization is then a [128,k]
  reciprocal plus per-head tensor_scalar multiplies - no broadcast matmul.
  Normalized O ([q, e] layout) is PE-transposed back to [e, q] tiles for
  the output projection (64 transposes of 128 rows each).
- Attention runs in (head-pair, query-half) units: energy+exp of unit u
  overlap the AV/normalize/transpose of unit u-1, so the Act engine's exp
  stream (the secondary bottleneck) is hidden behind PE work.
- Inputs/weights are loaded with one large DMA per tensor (the SP DMA
  queue was ~100% busy in the baseline with per-chunk DMAs); only qT/Wq
  are split in halves so the PE can start early.
"""

import sys

sys.path.insert(0, "/opt/trn_rl_repo")

from contextlib import ExitStack

import ml_dtypes
import numpy as np

import concourse.bass as bass  # noqa: F401
import concourse.tile as tile
from concourse import bacc, mybir
from concourse.bass_utils import run_bass_kernel_spmd

E = 1024          # embed dim
HEADS = 16
HD = 64           # head dim
B = 4
S = 2048
NCORES = 8
Q = (B * S) // NCORES  # query tokens per core
EC = E // 128     # embed chunks of 128
F32 = mybir.dt.float32
BF16 = mybir.dt.bfloat16
BF16NP = ml_dtypes.bfloat16


def build_program(Kpad):
    """Build the per-core Bass program (identical on all 8 cores)."""
    KTn = Kpad // 128
    nc = bacc.Bacc("TRN2", target_bir_lowering=False, debug=False,
                   num_devices=NCORES, dynamic_dma_scratch_size=2048)

    qT = nc.dram_tensor("qT", [E, Q], BF16, kind="ExternalInput").ap()
    kT = nc.dram_tensor("kT", [E, Kpad], BF16, kind="ExternalInput").ap()
    vT = nc.dram_tensor("vT", [E, Kpad], BF16, kind="ExternalInput").ap()
    wqT = nc.dram_tensor("wqT", [E, E], BF16, kind="ExternalInput").ap()
    wkT = nc.dram_tensor("wkT", [E, E], BF16, kind="ExternalInput").ap()
    wvT = nc.dram_tensor("wvT", [E, E], BF16, kind="ExternalInput").ap()
    woT = nc.dram_tensor("woT", [E, E], BF16, kind="ExternalInput").ap()
    bq2 = nc.dram_tensor("bq2", [128, EC], F32, kind="ExternalInput").ap()
    bk2 = nc.dram_tensor("bk2", [128, EC], F32, kind="ExternalInput").ap()
    bo2 = nc.dram_tensor("bo2", [128, EC], F32, kind="ExternalInput").ap()
    mb = nc.dram_tensor("mb", [128, KTn], F32, kind="ExternalInput").ap()
    ident = nc.dram_tensor("ident", [128, 128], BF16,
                           kind="ExternalInput").ap()
    ones16 = nc.dram_tensor("ones16", [128, 16], BF16,
                            kind="ExternalInput").ap()
    yT = nc.dram_tensor("yT", [E, Q], F32, kind="ExternalOutput").ap()

    # K-projection free-dim chunks (each must stay inside a 512-float bank)
    k_tail = Kpad - 1024 if Kpad > 1024 else 0

    with tile.TileContext(nc) as tc, ExitStack() as ctx:
        sml = ctx.enter_context(tc.tile_pool(name="sml", bufs=1))
        big = ctx.enter_context(tc.tile_pool(name="big", bufs=1))
        wo_pool = ctx.enter_context(tc.tile_pool(name="wop", bufs=1))
        inp_ctx = ExitStack()
        inp = inp_ctx.enter_context(tc.tile_pool(name="inp", bufs=1))

        # ---- PSUM pools: psE 2x[128,1024]=4 banks, psA 2x1=2, psT 2x1=2
        psE = ctx.enter_context(tc.tile_pool(name="psE", bufs=2, space="PSUM"))
        psA = ctx.enter_context(tc.tile_pool(name="psA", bufs=2, space="PSUM"))
        psT = ctx.enter_context(tc.tile_pool(name="psT", bufs=2, space="PSUM"))

        # ---- small constants (Pool-engine SWDGE queue) -----------------
        bq_s = sml.tile([128, EC], F32, name="bq_s")
        nc.gpsimd.dma_start(bq_s[:], bq2[:])
        bk_s = sml.tile([128, EC], F32, name="bk_s")
        nc.gpsimd.dma_start(bk_s[:], bk2[:])
        bo_s = sml.tile([128, EC], F32, name="bo_s")
        nc.gpsimd.dma_start(bo_s[:], bo2[:])
        mb_s = sml.tile([128, KTn], F32, name="mb_s")
        nc.gpsimd.dma_start(mb_s[:], mb[:])
        id_s = sml.tile([128, 128], BF16, name="id_s")
        nc.gpsimd.dma_start(id_s[:], ident[:])
        on_s = sml.tile([128, 16], BF16, name="on_s")
        nc.gpsimd.dma_start(on_s[:], ones16[:])

        # ---- big input DMAs (SP queue), in consumption order -----------
        # qT/wq halves so the first projection matmuls start early.
        qt_t, wq_t = [], []
        for h in (0, 1):
            t = inp.tile([128, 4, Q], BF16, name=f"qt{h}")
            nc.sync.dma_start(
                t[:], qT[:].rearrange("(c p) q -> p c q", p=128)[:, 4 * h:4 * h + 4, :])
            qt_t.append(t)
            w = inp.tile([128, 4, E], BF16, name=f"wq{h}")
            nc.sync.dma_start(
                w[:], wqT[:].rearrange("(c p) e -> p c e", p=128)[:, 4 * h:4 * h + 4, :])
            wq_t.append(w)
        kt_t = inp.tile([128, EC, Kpad], BF16, name="kt")
        nc.sync.dma_start(kt_t[:], kT[:].rearrange("(c p) k -> p c k", p=128))
        wk_t = inp.tile([128, EC, E], BF16, name="wk")
        nc.sync.dma_start(wk_t[:], wkT[:].rearrange("(c p) e -> p c e", p=128))
        vt_t = inp.tile([128, EC, Kpad], BF16, name="vt")
        nc.sync.dma_start(vt_t[:], vT[:].rearrange("(c p) k -> p c k", p=128))
        wv_t = inp.tile([128, EC, E], BF16, name="wv")
        nc.sync.dma_start(wv_t[:], wvT[:].rearrange("(c p) e -> p c e", p=128))
        wo_t = wo_pool.tile([128, EC, E], BF16, name="wo")
        nc.sync.dma_start(wo_t[:], woT[:].rearrange("(c p) e -> p c e", p=128))

        # ---- persistent SBUF tensors -----------------------------------
        QTs = [big.tile([128, Q], BF16, name=f"QT{m}") for m in range(EC)]
        KTs = [big.tile([128, Kpad], BF16, name=f"KT{m}") for m in range(EC)]
        VVs = [big.tile([128, HEADS * 65], BF16, name=f"VV{t}")
               for t in range(KTn)]
        OTs = [big.tile([128, Q], BF16, name=f"OT{m}") for m in range(EC)]

        # ---- phase A: Q projection (two k-halves) ----------------------
        # QT[m] = (Wq @ X^T + bq) rows m*128..m*128+128, bf16
        for h in (0, 1):
            for m in range(EC):
                ps = psE.tile([128, 1024], F32, tag="e", name=f"psq{h}_{m}")
                for n0 in (0, 512):
                    for kk in range(4):
                        nc.tensor.matmul(
                            ps[:, n0:n0 + 512],
                            wq_t[h][:, kk, m * 128:(m + 1) * 128],
                            qt_t[h][:, kk, n0:n0 + 512],
                            start=(kk == 0), stop=(kk == 3))
                if h == 0:
                    nc.vector.tensor_scalar_add(
                        QTs[m][:], ps[:], bq_s[:, m:m + 1])
                else:
                    with nc.allow_low_precision(reason="bf16 proj accum"):
                        nc.vector.tensor_add(QTs[m][:], QTs[m][:], ps[:])

        # ---- phase B: K projection -------------------------------------
        for m in range(EC):
            ps = psE.tile([128, 1024], F32, tag="e", name=f"psk{m}")
            for n0 in (0, 512):
                for kk in range(EC):
                    nc.tensor.matmul(
                        ps[:, n0:n0 + 512],
                        wk_t[:, kk, m * 128:(m + 1) * 128],
                        kt_t[:, kk, n0:n0 + 512],
                        start=(kk == 0), stop=(kk == EC - 1))
            nc.vector.tensor_scalar_add(
                KTs[m][:, 0:1024], ps[:], bk_s[:, m:m + 1])
            if k_tail:
                ps2 = psE.tile([128, 1024], F32, tag="e", name=f"pskt{m}")
                for kk in range(EC):
                    nc.tensor.matmul(
                        ps2[:, 0:k_tail],
                        wk_t[:, kk, m * 128:(m + 1) * 128],
                        kt_t[:, kk, 1024:Kpad],
                        start=(kk == 0), stop=(kk == EC - 1))
                nc.vector.tensor_scalar_add(
                    KTs[m][:, 1024:Kpad], ps2[:, 0:k_tail], bk_s[:, m:m + 1])

        # ---- phase C: V projection (no bias: folded into bo') ----------
        # VV[t][token, h*65+0:64] = V_h, col 64 = ones
        for t in range(KTn):
            ps = psE.tile([128, 1024], F32, tag="e", name=f"psv{t}")
            for n0 in (0, 512):
                for kk in range(EC):
                    nc.tensor.matmul(
                        ps[:, n0:n0 + 512],
                        vt_t[:, kk, t * 128:(t + 1) * 128],
                        wv_t[:, kk, n0:n0 + 512],
                        start=(kk == 0), stop=(kk == EC - 1))
            vv3 = VVs[t][:].rearrange("p (h e) -> p h e", e=65)
            ps3 = ps[:].rearrange("p (h d) -> p h d", d=64)
            nc.vector.tensor_copy(vv3[:, :, 0:64], ps3[:])
            nc.vector.tensor_copy(
                vv3[:, :, 64:65],
                on_s[:].rearrange("p (a b) -> p a b", b=1))

        # inputs (qT/kT/vT, wq/wk/wv) are dead once projections are done
        inp_ctx.close()

        # ---- attention: units of (head pair j, query half qh) ----------
        pp = ctx.enter_context(tc.tile_pool(name="pp", bufs=2))
        nrm = ctx.enter_context(tc.tile_pool(name="nrm", bufs=4))

        def emit_energy(j, qh):
            """Energy + exp for unit (j, qh); returns 9 P tiles
            [128 keys, 1024] bf16 (cols 0:512 head 2j, 512:1024 head 2j+1,
            over q-range qh*512..qh*512+512)."""
            ptiles = []
            for kt in range(KTn):
                pe = psE.tile([128, 1024], F32, tag="e", name=f"pe{j}_{qh}_{kt}")
                for hh in (0, 1):
                    off = hh * 64
                    nc.tensor.matmul(
                        pe[:, hh * 512:hh * 512 + 512],
                        KTs[j][off:off + 64, kt * 128:(kt + 1) * 128],
                        QTs[j][off:off + 64, qh * 512:qh * 512 + 512])
                pt = pp.tile([128, 1024], BF16, tag=f"P{qh}_{kt}",
                             name=f"pt{j}_{qh}_{kt}")
                nc.scalar.activation(
                    pt[:], pe[:], mybir.ActivationFunctionType.Exp,
                    bias=mb_s[:, kt:kt + 1], scale=0.125)
                ptiles.append(pt)
            return ptiles

        def emit_av(j, qh, ptiles, oj):
            """AV + normalize + transpose for unit (j, qh). Writes
            OTs[j][:, qc*128:(qc+1)*128] for qc in this half."""
            for pair in (0, 1):
                av = psA.tile([128, 260], F32, tag="a",
                              name=f"av{j}_{qh}_{pair}")
                # 4 output regions share one PSUM bank; start=True zeroes the
                # whole 2KB zero-region, so only the first matmul starts the
                # group and only the last stops it.
                nmm = KTn * 4
                for i, kt in enumerate(range(KTn)):
                    for qi in (0, 1):
                        for hh in (0, 1):
                            idx = kt * 4 + qi * 2 + hh
                            nc.tensor.matmul(
                                av[:, (qi * 2 + hh) * 65:(qi * 2 + hh + 1) * 65],
                                ptiles[kt][:, hh * 512 + (pair * 2 + qi) * 128:
                                           hh * 512 + (pair * 2 + qi) * 128 + 128],
                                VVs[kt][:, (2 * j + hh) * 65:(2 * j + hh + 1) * 65],
                                start=(idx == 0), stop=(idx == nmm - 1),
                                skip_group_check=True)
                av3 = av[:].rearrange("p (x c) -> p x c", c=65)
                rc = nrm.tile([128, 4], F32, tag="rc", name=f"rc{j}_{qh}_{pair}")
                nc.vector.reciprocal(
                    rc[:].rearrange("p (a b) -> p a b", b=1), av3[:, :, 64:65])
                for qi in (0, 1):
                    qc = qh * 4 + pair * 2 + qi
                    for hh in (0, 1):
                        i = qi * 2 + hh
                        nc.vector.tensor_scalar_mul(
                            oj[:, qc, hh * 64:hh * 64 + 64],
                            av[:, i * 65:i * 65 + 64], rc[:, i:i + 1])
                    tp = psT.tile([128, 128], BF16, tag="t",
                                  name=f"tp{j}_{qh}_{pair}_{qi}")
                    nc.tensor.transpose(tp[:], oj[:, qc, :], id_s[:])
                    nc.vector.tensor_copy(OTs[j][:, qc * 128:(qc + 1) * 128],
                                          tp[:])

        units = [(j, qh) for j in range(EC) for qh in (0, 1)]
        prev = None
        ojs = {}
        for j, qh in units:
            if qh == 0:
                ojs[j] = nrm.tile([128, EC, 128], BF16, tag="oj",
                                  name=f"oj{j}")
            ptiles = emit_energy(j, qh)
            if prev is not None:
                pj, pqh = prev
                emit_av(pj, pqh, prev_pt, ojs[pj])
            prev, prev_pt = (j, qh), ptiles
        emit_av(prev[0], prev[1], prev_pt, ojs[prev[0]])

        # ---- output projection Y^T = Wo @ O^T + bo' --------------------
        for m in range(EC):
            ps = psE.tile([128, 1024], F32, tag="e", name=f"psy{m}")
            for n0 in (0, 512):
                for k in range(EC):
                    nc.tensor.matmul(
                        ps[:, n0:n0 + 512],
                        wo_t[:, k, m * 128:(m + 1) * 128],
                        OTs[k][:, n0:n0 + 512],
                        start=(k == 0), stop=(k == EC - 1))
            yt = nrm.tile([128, Q], F32, tag="yt", name=f"yt{m}")
            nc.vector.tensor_scalar_add(yt[:], ps[:], bo_s[:, m:m + 1])
            nc.sync.dma_start(yT[m * 128:(m + 1) * 128, :], yt[:])

    nc.compile()
    return nc


_PROG_CACHE = {}


def _get_program(Kpad):
    if Kpad not in _PROG_CACHE:
        _PROG_CACHE[Kpad] = build_program(Kpad)
    return _PROG_CACHE[Kpad]


def prepare_inputs(query, keys, values, mask, Wq, bq, Wk, bk, Wv, bv, Wo, bo):
    """Host-side sharding/layout prep. Returns (Kpad, in_maps)."""
    f32 = np.float32
    query = np.asarray(query, f32)
    keys = np.asarray(keys, f32)
    values = np.asarray(values, f32)
    mask = np.asarray(mask)

    idxs = [np.nonzero(mask[b] != 0)[0] for b in range(B)]
    nmax = max(len(i) for i in idxs)
    Kpad = max(256, ((max(nmax, 1) + 127) // 128) * 128)
    KTn = Kpad // 128

    kTb = np.zeros((B, E, Kpad), BF16NP)
    vTb = np.zeros((B, E, Kpad), BF16NP)
    mbb = np.full((B, Kpad), -1e9, f32)
    for b in range(B):
        n = len(idxs[b])
        kTb[b, :, :n] = keys[b][idxs[b]].T.astype(BF16NP)
        vTb[b, :, :n] = values[b][idxs[b]].T.astype(BF16NP)
        mbb[b, :n] = 0.0
    mb2 = np.ascontiguousarray(mbb.reshape(B, KTn, 128).transpose(0, 2, 1))

    WqT = np.ascontiguousarray(np.asarray(Wq, f32).T.astype(BF16NP))
    WkT = np.ascontiguousarray(np.asarray(Wk, f32).T.astype(BF16NP))
    WvT = np.ascontiguousarray(np.asarray(Wv, f32).T.astype(BF16NP))
    WoT = np.ascontiguousarray(np.asarray(Wo, f32).T.astype(BF16NP))
    bq2 = np.ascontiguousarray(np.asarray(bq, f32).reshape(EC, 128).T)
    bk2 = np.ascontiguousarray(np.asarray(bk, f32).reshape(EC, 128).T)
    # fold V bias through the output projection: y += (Wo @ bv + bo)
    bo_f = np.asarray(bo, f32) + np.asarray(Wo, f32) @ np.asarray(bv, f32)
    bo2 = np.ascontiguousarray(bo_f.reshape(EC, 128).T)
    ident = np.eye(128, dtype=BF16NP)
    ones16 = np.ones((128, 16), BF16NP)

    in_maps = []
    for c in range(NCORES):
        b, h = c // 2, c % 2
        in_maps.append(dict(
            qT=np.ascontiguousarray(
                query[b, h * Q:(h + 1) * Q, :].T.astype(BF16NP)),
            kT=kTb[b], vT=vTb[b], mb=mb2[b],
            wqT=WqT, wkT=WkT, wvT=WvT, woT=WoT,
            bq2=bq2, bk2=bk2, bo2=bo2,
            ident=ident, ones16=ones16,
        ))
    return Kpad, in_maps


def kernel(query, keys, values, mask, Wq, bq, Wk, bk, Wv, bv, Wo, bo):
    Kpad, in_maps = prepare_inputs(query, keys, values, mask,
                                   Wq, bq, Wk, bk, Wv, bv, Wo, bo)
    nc = _get_program(Kpad)
    res = run_bass_kernel_spmd(nc, in_maps, list(range(NCORES)))
    out = np.empty((B, S, E), np.float32)
    for c in range(NCORES):
        b, h = c // 2, c % 2
        out[b, h * Q:(h + 1) * Q, :] = res.results[c]["yT"].T
    return out


# revision 11
# speedup vs baseline: 1.1731x; 1.1385x over previous
"""Multi-head attention on 8 TRN2 NeuronCores (Bass/Tile).

Sharding: core c handles batch b = c//2 and query-half h = c%2 (1024 query
tokens), all 16 heads. K/V projections are per-batch and duplicated across
the two cores sharing a batch; no cross-core communication.

Design notes (v3):
- All matmul operands are bf16 (1 PE cycle/row at any output width, vs
  fp32r's 4x penalty below 256), halving DMA traffic as well. PSUM stays
  fp32.
- Keys are compacted on the host using the 0/1 key mask (masked keys
  dropped, padded to a multiple of 128, Kpad). Pad positions are killed by
  a -1e9 per-partition bias folded into the Exp activation.
- The V-projection bias is folded into the output-projection bias on the
  host (bo' = bo + Wo @ bv), since softmax weights sum to 1.
- Energy is computed transposed ([key, query] tiles). AV is computed as
  out[q-tile, 65] accumulating over key tiles: each head's V tile carries
  an extra ones column, so column 64 of the PSUM output is the softmax
  denominator, a per-partition scalar. Normalization is then a [128,k]
  reciprocal plus per-head tensor_scalar multiplies - no broadcast matmul.
  Normalized O ([q, e] layout) is PE-transposed back to [e, q] tiles for
  the output projection (64 transposes of 128 rows each).
- Attention runs in (head-pair, query-half) units: energy+exp of unit u
  overlap the AV/normalize/transpose of unit u-1, so the Act engine's exp
  stream (the secondary bottleneck) is hidden behind PE work.
- Inputs/weights are loaded with one large DMA per tensor (the SP DMA
  queue was ~100% busy in the baseline with per-chunk DMAs); only qT/Wq
  are split in halves so the PE can start early.
"""

import sys

sys.path.insert(0, "/opt/trn_rl_repo")

from contextlib import ExitStack

import ml_dtypes
import numpy as np

import concourse.bass as bass  # noqa: F401
import concourse.tile as tile
from concourse import bacc, mybir
from concourse.bass_utils import run_bass_kernel_spmd

E = 1024          # embed dim
HEADS = 16
HD = 64           # head dim
B = 4
S = 2048
NCORES = 8
Q = (B * S) // NCORES  # query tokens per core
EC = E // 128     # embed chunks of 128
F32 = mybir.dt.float32
BF16 = mybir.dt.bfloat16
BF16NP = ml_dtypes.bfloat16


def build_program(Kpad):
    """Build the per-core Bass program (identical on all 8 cores)."""
    KTn = Kpad // 128
    nc = bacc.Bacc("TRN2", target_bir_lowering=False, debug=False,
                   num_devices=NCORES, dynamic_dma_scratch_size=2048)

    qT = nc.dram_tensor("qT", [E, Q], BF16, kind="ExternalInput").ap()
    kT = nc.dram_tensor("kT", [E, Kpad], BF16, kind="ExternalInput").ap()
    vT = nc.dram_tensor("vT", [E, Kpad], BF16, kind="ExternalInput").ap()
    wqT = nc.dram_tensor("wqT", [E, E], BF16, kind="ExternalInput").ap()
    wkT = nc.dram_tensor("wkT", [E, E], BF16, kind="ExternalInput").ap()
    wvT = nc.dram_tensor("wvT", [E, E], BF16, kind="ExternalInput").ap()
    woT = nc.dram_tensor("woT", [E, E], BF16, kind="ExternalInput").ap()
    bq2 = nc.dram_tensor("bq2", [128, EC], F32, kind="ExternalInput").ap()
    bk2 = nc.dram_tensor("bk2", [128, EC], F32, kind="ExternalInput").ap()
    bo2 = nc.dram_tensor("bo2", [128, EC], F32, kind="ExternalInput").ap()
    mb = nc.dram_tensor("mb", [128, KTn], F32, kind="ExternalInput").ap()
    ident = nc.dram_tensor("ident", [128, 128], BF16,
                           kind="ExternalInput").ap()
    yT = nc.dram_tensor("yT", [E, Q], F32, kind="ExternalOutput").ap()

    # K-projection free-dim chunks (each must stay inside a 512-float bank)
    k_tail = Kpad - 1024 if Kpad > 1024 else 0

    with tile.TileContext(nc) as tc, ExitStack() as ctx:
        sml = ctx.enter_context(tc.tile_pool(name="sml", bufs=1))
        big = ctx.enter_context(tc.tile_pool(name="big", bufs=1))
        wo_pool = ctx.enter_context(tc.tile_pool(name="wop", bufs=1))
        inp_ctx = ExitStack()
        inp = inp_ctx.enter_context(tc.tile_pool(name="inp", bufs=1))

        # ---- PSUM pools: psE 2x[128,1024]=4 banks, psA 2x1=2, psT 2x1=2
        psE = ctx.enter_context(tc.tile_pool(name="psE", bufs=2, space="PSUM"))
        psA = ctx.enter_context(tc.tile_pool(name="psA", bufs=2, space="PSUM"))
        psT = ctx.enter_context(tc.tile_pool(name="psT", bufs=2, space="PSUM"))

        # ---- small constants first (tiny, unblock everything) ----------
        bq_s = sml.tile([128, EC], F32, name="bq_s")
        nc.sync.dma_start(bq_s[:], bq2[:])
        bk_s = sml.tile([128, EC], F32, name="bk_s")
        nc.sync.dma_start(bk_s[:], bk2[:])
        bo_s = sml.tile([128, EC], F32, name="bo_s")
        nc.sync.dma_start(bo_s[:], bo2[:])
        mb_s = sml.tile([128, KTn], F32, name="mb_s")
        nc.sync.dma_start(mb_s[:], mb[:])
        id_s = sml.tile([128, 128], BF16, name="id_s")
        nc.sync.dma_start(id_s[:], ident[:])

        # ---- big input DMAs (SP queue), in consumption order -----------
        # qT/wq are DMA'd per 128-row chunk so the first matmuls start early.
        qt_t, wq_t = [], []
        qv = qT[:].rearrange("(c p) q -> p c q", p=128)
        wv = wqT[:].rearrange("(c p) e -> p c e", p=128)
        for h in (0, 1):
            t = inp.tile([128, 4, Q], BF16, name=f"qt{h}")
            w = inp.tile([128, 4, E], BF16, name=f"wq{h}")
            for kk in range(4):
                nc.sync.dma_start(t[:, kk, :], qv[:, 4 * h + kk, :])
                nc.sync.dma_start(w[:, kk, :], wv[:, 4 * h + kk, :])
            qt_t.append(t)
            wq_t.append(w)
        kt_t = inp.tile([128, EC, Kpad], BF16, name="kt")
        nc.sync.dma_start(kt_t[:], kT[:].rearrange("(c p) k -> p c k", p=128))
        wk_t = inp.tile([128, EC, E], BF16, name="wk")
        nc.sync.dma_start(wk_t[:], wkT[:].rearrange("(c p) e -> p c e", p=128))
        vt_t = inp.tile([128, EC, Kpad], BF16, name="vt")
        nc.sync.dma_start(vt_t[:], vT[:].rearrange("(c p) k -> p c k", p=128))
        wv_t = inp.tile([128, EC, E], BF16, name="wv")
        nc.sync.dma_start(wv_t[:], wvT[:].rearrange("(c p) e -> p c e", p=128))
        wo_t = wo_pool.tile([128, EC, E], BF16, name="wo")
        nc.sync.dma_start(wo_t[:], woT[:].rearrange("(c p) e -> p c e", p=128))

        # ---- persistent SBUF tensors -----------------------------------
        QTs = [big.tile([128, Q], BF16, name=f"QT{m}") for m in range(EC)]
        KTs = [big.tile([128, Kpad], BF16, name=f"KT{m}") for m in range(EC)]
        VVs = [big.tile([128, HEADS * 65], BF16, name=f"VV{t}")
               for t in range(KTn)]
        OTs = [big.tile([128, Q], BF16, name=f"OT{m}") for m in range(EC)]

        # ---- phase A: Q projection (two k-halves) ----------------------
        # QT[m] = (Wq @ X^T + bq) rows m*128..m*128+128, bf16
        for h in (0, 1):
            for m in range(EC):
                ps = psE.tile([128, 1024], F32, tag="e", name=f"psq{h}_{m}")
                for n0 in (0, 512):
                    for kk in range(4):
                        nc.tensor.matmul(
                            ps[:, n0:n0 + 512],
                            wq_t[h][:, kk, m * 128:(m + 1) * 128],
                            qt_t[h][:, kk, n0:n0 + 512],
                            start=(kk == 0), stop=(kk == 3))
                if h == 0:
                    nc.vector.tensor_scalar_add(
                        QTs[m][:], ps[:], bq_s[:, m:m + 1])
                else:
                    with nc.allow_low_precision(reason="bf16 proj accum"):
                        nc.vector.tensor_add(QTs[m][:], QTs[m][:], ps[:])

        # ---- phase B: K projection -------------------------------------
        for m in range(EC):
            ps = psE.tile([128, 1024], F32, tag="e", name=f"psk{m}")
            for n0 in (0, 512):
                for kk in range(EC):
                    nc.tensor.matmul(
                        ps[:, n0:n0 + 512],
                        wk_t[:, kk, m * 128:(m + 1) * 128],
                        kt_t[:, kk, n0:n0 + 512],
                        start=(kk == 0), stop=(kk == EC - 1))
            nc.vector.tensor_scalar_add(
                KTs[m][:, 0:1024], ps[:], bk_s[:, m:m + 1])
            if k_tail:
                ps2 = psE.tile([128, 1024], F32, tag="e", name=f"pskt{m}")
                for kk in range(EC):
                    nc.tensor.matmul(
                        ps2[:, 0:k_tail],
                        wk_t[:, kk, m * 128:(m + 1) * 128],
                        kt_t[:, kk, 1024:Kpad],
                        start=(kk == 0), stop=(kk == EC - 1))
                nc.vector.tensor_scalar_add(
                    KTs[m][:, 1024:Kpad], ps2[:, 0:k_tail], bk_s[:, m:m + 1])

        # ---- phase C: V projection (no bias: folded into bo') ----------
        # VV[t][token, h*65+0:64] = V_h, col 64 = ones
        for t in range(KTn):
            ps = psE.tile([128, 1024], F32, tag="e", name=f"psv{t}")
            for n0 in (0, 512):
                for kk in range(EC):
                    nc.tensor.matmul(
                        ps[:, n0:n0 + 512],
                        vt_t[:, kk, t * 128:(t + 1) * 128],
                        wv_t[:, kk, n0:n0 + 512],
                        start=(kk == 0), stop=(kk == EC - 1))
            vv3 = VVs[t][:].rearrange("p (h e) -> p h e", e=65)
            ps3 = ps[:].rearrange("p (h d) -> p h d", d=64)
            nc.vector.tensor_copy(vv3[:, :, 0:64], ps3[:])
            nc.vector.memset(vv3[:, :, 64:65], 1.0)

        # inputs (qT/kT/vT, wq/wk/wv) are dead once projections are done
        inp_ctx.close()

        # ---- attention: units of (head pair j, query half qh) ----------
        pp = ctx.enter_context(tc.tile_pool(name="pp", bufs=2))
        nrm = ctx.enter_context(tc.tile_pool(name="nrm", bufs=4))

        def emit_energy(j, qh):
            """Energy + exp for unit (j, qh); returns 9 P tiles
            [128 keys, 1024] bf16 (cols 0:512 head 2j, 512:1024 head 2j+1,
            over q-range qh*512..qh*512+512)."""
            ptiles = []
            for kt in range(KTn):
                pe = psE.tile([128, 1024], F32, tag="e", name=f"pe{j}_{qh}_{kt}")
                for hh in (0, 1):
                    off = hh * 64
                    nc.tensor.matmul(
                        pe[:, hh * 512:hh * 512 + 512],
                        KTs[j][off:off + 64, kt * 128:(kt + 1) * 128],
                        QTs[j][off:off + 64, qh * 512:qh * 512 + 512])
                pt = pp.tile([128, 1024], BF16, tag=f"P{qh}_{kt}",
                             name=f"pt{j}_{qh}_{kt}")
                nc.scalar.activation(
                    pt[:], pe[:], mybir.ActivationFunctionType.Exp,
                    bias=mb_s[:, kt:kt + 1], scale=0.125)
                ptiles.append(pt)
            return ptiles

        def emit_av(j, qh, ptiles, oj):
            """AV + normalize + transpose for unit (j, qh). Writes
            OTs[j][:, qc*128:(qc+1)*128] for qc in this half."""
            for pair in (0, 1):
                av = psA.tile([128, 260], F32, tag="a",
                              name=f"av{j}_{qh}_{pair}")
                # 4 output regions share one PSUM bank; start=True zeroes the
                # whole 2KB zero-region, so only the first matmul starts the
                # group and only the last stops it.
                nmm = KTn * 4
                for i, kt in enumerate(range(KTn)):
                    for qi in (0, 1):
                        for hh in (0, 1):
                            idx = kt * 4 + qi * 2 + hh
                            nc.tensor.matmul(
                                av[:, (qi * 2 + hh) * 65:(qi * 2 + hh + 1) * 65],
                                ptiles[kt][:, hh * 512 + (pair * 2 + qi) * 128:
                                           hh * 512 + (pair * 2 + qi) * 128 + 128],
                                VVs[kt][:, (2 * j + hh) * 65:(2 * j + hh + 1) * 65],
                                start=(idx == 0), stop=(idx == nmm - 1),
                                skip_group_check=True)
                av3 = av[:].rearrange("p (x c) -> p x c", c=65)
                rc = nrm.tile([128, 4], F32, tag="rc", name=f"rc{j}_{qh}_{pair}")
                nc.vector.reciprocal(
                    rc[:].rearrange("p (a b) -> p a b", b=1), av3[:, :, 64:65])
                for qi in (0, 1):
                    qc = qh * 4 + pair * 2 + qi
                    for hh in (0, 1):
                        i = qi * 2 + hh
                        nc.vector.tensor_scalar_mul(
                            oj[:, qc, hh * 64:hh * 64 + 64],
                            av[:, i * 65:i * 65 + 64], rc[:, i:i + 1])
                    tp = psT.tile([128, 128], BF16, tag="t",
                                  name=f"tp{j}_{qh}_{pair}_{qi}")
                    nc.tensor.transpose(tp[:], oj[:, qc, :], id_s[:])
                    nc.vector.tensor_copy(OTs[j][:, qc * 128:(qc + 1) * 128],
                                          tp[:])

        units = [(j, qh) for j in range(EC) for qh in (0, 1)]
        prev = None
        ojs = {}
        for j, qh in units:
            if qh == 0:
                ojs[j] = nrm.tile([128, EC, 128], BF16, tag="oj",
                                  name=f"oj{j}")
            ptiles = emit_energy(j, qh)
            if prev is not None:
                pj, pqh = prev
                emit_av(pj, pqh, prev_pt, ojs[pj])
            prev, prev_pt = (j, qh), ptiles
        emit_av(prev[0], prev[1], prev_pt, ojs[prev[0]])

        # ---- output projection Y^T = Wo @ O^T + bo' --------------------
        for m in range(EC):
            ps = psE.tile([128, 1024], F32, tag="e", name=f"psy{m}")
            for n0 in (0, 512):
                for k in range(EC):
                    nc.tensor.matmul(
                        ps[:, n0:n0 + 512],
                        wo_t[:, k, m * 128:(m + 1) * 128],
                        OTs[k][:, n0:n0 + 512],
                        start=(k == 0), stop=(k == EC - 1))
            yt = nrm.tile([128, Q], F32, tag="yt", name=f"yt{m}")
            nc.vector.tensor_scalar_add(yt[:], ps[:], bo_s[:, m:m + 1])
            nc.sync.dma_start(yT[m * 128:(m + 1) * 128, :], yt[:])

    nc.compile()
    return nc


_PROG_CACHE = {}


def _get_program(Kpad):
    if Kpad not in _PROG_CACHE:
        _PROG_CACHE[Kpad] = build_program(Kpad)
    return _PROG_CACHE[Kpad]


def prepare_inputs(query, keys, values, mask, Wq, bq, Wk, bk, Wv, bv, Wo, bo):
    """Host-side sharding/layout prep. Returns (Kpad, in_maps)."""
    f32 = np.float32
    query = np.asarray(query, f32)
    keys = np.asarray(keys, f32)
    values = np.asarray(values, f32)
    mask = np.asarray(mask)

    idxs = [np.nonzero(mask[b] != 0)[0] for b in range(B)]
    nmax = max(len(i) for i in idxs)
    Kpad = max(256, ((max(nmax, 1) + 127) // 128) * 128)
    KTn = Kpad // 128

    kTb = np.zeros((B, E, Kpad), BF16NP)
    vTb = np.zeros((B, E, Kpad), BF16NP)
    mbb = np.full((B, Kpad), -1e9, f32)
    for b in range(B):
        n = len(idxs[b])
        kTb[b, :, :n] = keys[b][idxs[b]].T.astype(BF16NP)
        vTb[b, :, :n] = values[b][idxs[b]].T.astype(BF16NP)
        mbb[b, :n] = 0.0
    mb2 = np.ascontiguousarray(mbb.reshape(B, KTn, 128).transpose(0, 2, 1))

    WqT = np.ascontiguousarray(np.asarray(Wq, f32).T.astype(BF16NP))
    WkT = np.ascontiguousarray(np.asarray(Wk, f32).T.astype(BF16NP))
    WvT = np.ascontiguousarray(np.asarray(Wv, f32).T.astype(BF16NP))
    WoT = np.ascontiguousarray(np.asarray(Wo, f32).T.astype(BF16NP))
    bq2 = np.ascontiguousarray(np.asarray(bq, f32).reshape(EC, 128).T)
    bk2 = np.ascontiguousarray(np.asarray(bk, f32).reshape(EC, 128).T)
    # fold V bias through the output projection: y += (Wo @ bv + bo)
    bo_f = np.asarray(bo, f32) + np.asarray(Wo, f32) @ np.asarray(bv, f32)
    bo2 = np.ascontiguousarray(bo_f.reshape(EC, 128).T)
    ident = np.eye(128, dtype=BF16NP)

    in_maps = []
    for c in range(NCORES):
        b, h = c // 2, c % 2
        in_maps.append(dict(
            qT=np.ascontiguousarray(
                query[b, h * Q:(h + 1) * Q, :].T.astype(BF16NP)),
            kT=kTb[b], vT=vTb[b], mb=mb2[b],
            wqT=WqT, wkT=WkT, wvT=WvT, woT=WoT,
            bq2=bq2, bk2=bk2, bo2=bo2,
            ident=ident,
        ))
    return Kpad, in_maps


def kernel(query, keys, values, mask, Wq, bq, Wk, bk, Wv, bv, Wo, bo):
    Kpad, in_maps = prepare_inputs(query, keys, values, mask,
                                   Wq, bq, Wk, bk, Wv, bv, Wo, bo)
    nc = _get_program(Kpad)
    res = run_bass_kernel_spmd(nc, in_maps, list(range(NCORES)))
    out = np.empty((B, S, E), np.float32)
    for c in range(NCORES):
        b, h = c // 2, c % 2
        out[b, h * Q:(h + 1) * Q, :] = res.results[c]["yT"].T
    return out
